# revision 14
# baseline (speedup 1.0000x reference)
"""Branched feed-forward (4-phase MoE-style FF) on 8 Trainium2 NeuronCores.

Reference computation (B=32, S=1024, D=1024, P=4, F=4096):
    xs = x.reshape(B, P, S//P, D)              # static contiguous phase split
    h  = relu(xs @ W1[p] + b1[p])              # per-phase FF, D -> F
    y  = h @ W2[p] + b2[p]                     # F -> D
    out = y.reshape(B, S, D)

Sharding: 8 cores = 4 phases x 2 F-halves (expert parallel + FF-width
parallel).  Core c handles phase p = c//2, F-half fh = c%2: it computes a
partial y (contraction over its half of F) for ALL 8192 tokens of its
phase.  Host sums the two partials per phase and adds b2 (cheap numpy).

Per-core kernel (all weights SBUF-resident, bf16 matmuls / fp32 PSUM):
    FF1(tb): h[ft, :] = relu( sum_dc W1c[ft][dc].T @ x[dc, :] + b1[ft] )
    FF2(tb): y[dt, :] = sum_fc W2c[fc][:, dt].T @ h[fc, :]

Weights are DMA'd in 2KB-per-partition chunks (one per 128-wide output
tile) so the first matmuls start as soon as chunk 0 + x-block 0 land,
hiding the ~8MB weight load behind compute instead of serializing it at
NEFF start.  The x block DMA is split across DC chunks to spread it over
multiple DMA queues (a single queue only gets ~1/11 of HBM bandwidth
while the weight chunks stream in).

Measured on this hardware (8 cores sustained): the PE streams at
~2.0GHz (P0 power-state downclock; not the 2.4GHz nominal), so the
per-core floor is 4096 MMs x (512cyc/2.0GHz + ~10ns dispatch) ~ 1.09ms.
bf16 and fp32r stream at the same 1 cycle/row (fp32r is single-pass at
N>=256), so the win over the fp32r baseline comes from halved LDWEIGHTS
/ DMA bytes, tt=512 (half the MM dispatch overhead), and the overlapped
weight load.  fp8 DoubleRow (0.5 cyc/row) fails accuracy: measured
5.0e-2 final rel err vs the 2e-2 gate (Gaussian data, e4m3 has ~3.5%
RMS quantization); the 3-matmul error-split costs 1.7x bf16 cycles.
An FF1(tb+1)/FF2(tb) skewed pipeline measured 11us WORSE (strict-FIFO
ACT queue head-blocks on the skewed WAR, backpressuring PSUM rotation),
so the layers run in natural order.

Matmul dtype MM_DT: "bfloat16" (~3.2e-3 rel err on the graded inputs)
vs gate 2e-2.
"""

import numpy as np

import concourse.bacc as bacc
import concourse.mybir as mybir
import concourse.tile as tile
from concourse.bass import ts

# Problem dims (hardcoded per contest contract)
B, S, D = 32, 1024, 1024
P, F = 4, 4096
N_CORES = 8

# Per-core dims
FH = F // 2          # F half per core = 2048
T = B * (S // P)     # tokens per phase = 8192
DC = D // 128        # 8 contraction chunks for FF1 / out tiles for FF2
FT = FH // 128       # 16 out tiles for FF1 / contraction chunks for FF2

# Tunables (defaults = the graded configuration)
MM_DT = "bfloat16"   # matmul dtype: "bfloat16" | "float32r"
TT = 512             # token block (matmul moving free dim)

F32 = mybir.dt.float32


def build_bass(reps=1, loop_reps=1, mm_dt=None, tt=None, skew=False, mono=False):
    """Build the per-core Bass program.

    `reps` repeats the compute sweep by instruction duplication; `loop_reps`
    repeats it via a hardware For_i loop (no code growth).  Both are timing
    aids for test.py (slope between rep counts isolates on-device time);
    the graded kernel uses reps=1, loop_reps=1."""
    mm_dt = MM_DT if mm_dt is None else mm_dt
    tt = TT if tt is None else tt
    DT = getattr(mybir.dt, mm_dt)
    tb_n = T // tt

    nc = bacc.Bacc(None, target_bir_lowering=False)

    # Host pre-permutes everything so every DMA line is one contiguous
    # per-partition chunk (x: tt*esz per dc chunk, w1/w2: 2KB, y: tt*4B).
    x_d = nc.dram_tensor("x", [tb_n, 128, DC, tt], DT, kind="ExternalInput")
    w1_d = nc.dram_tensor("w1", [FT, 128, DC, 128], DT, kind="ExternalInput")
    w2_d = nc.dram_tensor("w2", [FT, 128, D], DT, kind="ExternalInput")
    b1_d = nc.dram_tensor("b1", [128, FT], F32, kind="ExternalInput")
    y_d = nc.dram_tensor("y", [tb_n, DC, 128, tt], F32, kind="ExternalOutput")

    with tile.TileContext(nc) as tc:
        with (
            tc.tile_pool(name="weights", bufs=1) as wpool,
            tc.tile_pool(name="xin", bufs=3) as xpool,
            tc.tile_pool(name="hbuf", bufs=2) as hpool,
            tc.tile_pool(name="yout", bufs=4) as ypool,
            tc.tile_pool(name="psum", bufs=8, space="PSUM") as psum,
        ):
            x_first = {}
            if mono:
                # Baseline-like dependency structure: one tile per weight
                # matrix, one DMA per x block (strided gather per partition).
                w1all = wpool.tile([128, FT, DC, 128], DT, tag="w1")
                nc.sync.dma_start(w1all[:], w1_d.transpose([1, 0, 2, 3]))
                w2all = wpool.tile([128, FT, D], DT, tag="w2")
                nc.sync.dma_start(w2all[:], w2_d.transpose([1, 0, 2]))
                b1_s = wpool.tile([128, FT], F32, tag="b1")
                nc.sync.dma_start(b1_s[:], b1_d[:])
                w1c = [w1all[:, ft] for ft in range(FT)]
                w2c = [w2all[:, fc] for fc in range(FT)]
            else:
                # DMA issue order = FIFO position per queue, so order by
                # criticality: x(0) chunks and w1 chunk 0 (both needed by the
                # first matmul group) are split across queues and issued first;
                # 33 weight chunks ahead of x(0) would stall the PE ~20us.
                xf0 = wpool.tile([128, DC, tt], DT, tag="x0", name="xf0")
                for dc in range(DC):
                    nc.sync.dma_start(xf0[:, dc, :], x_d[0, :, dc, :])
                x_first[0] = xf0
                w1c = [
                    wpool.tile([128, DC, 128], DT, tag=f"w1_{ft}", name=f"w1c{ft}")
                    for ft in range(FT)
                ]
                for dc in range(DC):
                    nc.sync.dma_start(w1c[0][:, dc, :], w1_d[0, :, dc, :])
                if tb_n > 1:
                    xf1 = wpool.tile([128, DC, tt], DT, tag="x1", name="xf1")
                    for dc in range(DC):
                        nc.sync.dma_start(xf1[:, dc, :], x_d[1, :, dc, :])
                    x_first[1] = xf1
                b1_s = wpool.tile([128, FT], F32, tag="b1")
                nc.sync.dma_start(b1_s[:], b1_d[:])
                for ft in range(1, FT):
                    nc.sync.dma_start(w1c[ft][:], w1_d[ft])
                w2c = []
                for fc in range(FT):
                    w = wpool.tile([128, D], DT, tag=f"w2_{fc}", name=f"w2c{fc}")
                    nc.sync.dma_start(w[:], w2_d[fc])
                    w2c.append(w)

            def ff2(h_t, tb):
                for dt_ in range(DC):
                    ps = psum.tile([128, tt], F32, tag="ps")
                    for fc in range(FT):
                        nc.tensor.matmul(
                            ps[:],
                            w2c[fc][:, ts(dt_, 128)],
                            h_t[:, fc, :],
                            start=(fc == 0),
                            stop=(fc == FT - 1),
                        )
                    y_t = ypool.tile([128, tt], F32, tag="y")
                    nc.vector.tensor_copy(y_t[:], ps[:])
                    nc.sync.dma_start(y_d[tb, dt_], y_t[:])

            def sweep():
                prev = None
                for tb in [t for _ in range(reps) for t in range(tb_n)]:
                    if tb in x_first:
                        x_t = x_first[tb]
                    elif mono:
                        x_t = xpool.tile([128, DC, tt], DT, tag="x")
                        nc.sync.dma_start(x_t[:], x_d[tb])
                    else:
                        x_t = xpool.tile([128, DC, tt], DT, tag="x")
                        for dc in range(DC):
                            nc.sync.dma_start(x_t[:, dc, :], x_d[tb, :, dc, :])

                    h_t = hpool.tile([128, FT, tt], DT, tag="h")
                    for ft in range(FT):
                        ps = psum.tile([128, tt], F32, tag="ps")
                        for dc in range(DC):
                            nc.tensor.matmul(
                                ps[:],
                                w1c[ft][:, dc, :],
                                x_t[:, dc, :],
                                start=(dc == 0),
                                stop=(dc == DC - 1),
                            )
                        nc.scalar.activation(
                            h_t[:, ft, :],
                            ps[:],
                            mybir.ActivationFunctionType.Relu,
                            bias=b1_s[:, ft : ft + 1],
                        )
                    if skew:
                        if prev is not None:
                            ff2(*prev)
                        prev = (h_t, tb)
                    else:
                        ff2(h_t, tb)
                if skew:
                    ff2(*prev)

            if loop_reps > 1:
                with tc.For_i(0, loop_reps, 1):
                    sweep()
            else:
                sweep()

    nc.compile()
    return nc


def _np_dt(mm_dt=None):
    return mybir.dt.np(getattr(mybir.dt, MM_DT if mm_dt is None else mm_dt))


def _shard_inputs(x, W1, b1, W2, mm_dt=None, tt=None):
    """Build the 8 per-core input maps. Core c: phase c//2, F-half c%2."""
    tt = TT if tt is None else tt
    tb_n = T // tt
    np_dt = _np_dt(mm_dt)
    in_maps = []
    xt_by_phase = {}
    for c in range(N_CORES):
        p, fh = divmod(c, 2)
        if p not in xt_by_phase:  # both F-half cores of a phase share x
            xs = np.ascontiguousarray(x.reshape(B, P, S // P, D)[:, p])
            xt_by_phase[p] = np.ascontiguousarray(
                xs.reshape(tb_n, tt, DC, 128).transpose(0, 3, 2, 1)  # [tbn,128,DC,tt]
            ).astype(np_dt)
        xt = xt_by_phase[p]
        # w1: [FT, 128, DC, 128]; w1[ft, part, dc, j] = W1[p][dc*128+part, fh*FH+ft*128+j]
        w1 = (
            W1[p][:, fh * FH : (fh + 1) * FH]
            .reshape(DC, 128, FT, 128)
            .transpose(2, 1, 0, 3)
        )
        # w2: [FT, 128, D]; w2[fc, part, :] = W2[p][fh*FH+fc*128+part, :]
        w2 = W2[p][fh * FH : (fh + 1) * FH, :].reshape(FT, 128, D)
        b1c = b1[p][fh * FH : (fh + 1) * FH].reshape(FT, 128).T
        in_maps.append(
            {
                "x": xt,
                "w1": np.ascontiguousarray(w1).astype(np_dt),
                "w2": np.ascontiguousarray(w2).astype(np_dt),
                "b1": np.ascontiguousarray(b1c).astype(np.float32),
            }
        )
    return in_maps


def _unshard_outputs(results, b2, tt=None):
    """results: list of 8 dicts with 'y' [tb_n,DC,128,tt] partial sums."""
    tt = TT if tt is None else tt
    y = np.empty((B, P, S // P, D), dtype=np.float32)
    for p in range(P):
        ya = results[2 * p]["y"]
        yb = results[2 * p + 1]["y"]
        # [tbn,DC,128,tt] -> [tbn,tt,DC,128] -> [T, D]
        yp = (ya + yb).transpose(0, 3, 1, 2).reshape(T, D) + b2[p][None, :]
        y[:, p] = yp.reshape(B, S // P, D)
    return y.reshape(B, S, D)


# ---------------------------------------------------------------------------
# Compile-once PJRT runner (mirrors concourse.bass2jax.run_bass_via_pjrt but
# caches the sharded executable so repeat kernel() calls skip re-tracing).

_RUNNER = None


def _make_runner():
    import jax
    from jax.sharding import Mesh, PartitionSpec
    from jax.experimental.shard_map import shard_map
    from concourse.bass2jax import (
        _bass_exec_p,
        install_neuronx_cc_hook,
        partition_id_tensor,
    )

    nc = build_bass()
    install_neuronx_cc_hook()

    partition_name = nc.partition_id_tensor.name if nc.partition_id_tensor else None

    in_names, out_names, out_avals = [], [], []
    for alloc in nc.m.functions[0].allocations:
        if not isinstance(alloc, mybir.MemoryLocationSet):
            continue
        name = alloc.memorylocations[0].name
        if alloc.kind == "ExternalInput":
            if name != partition_name:
                in_names.append(name)
        elif alloc.kind == "ExternalOutput":
            out_names.append(name)
            out_avals.append(
                jax.core.ShapedArray(
                    tuple(alloc.tensor_shape), mybir.dt.np(alloc.dtype)
                )
            )
    n_params = len(in_names)
    all_in_names = list(in_names) + list(out_names)
    if partition_name is not None:
        all_in_names.append(partition_name)

    def _body(*args):
        operands = list(args)
        if partition_name is not None:
            operands.append(partition_id_tensor())
        outs = _bass_exec_p.bind(
            *operands,
            out_avals=tuple(out_avals),
            in_names=tuple(all_in_names),
            out_names=tuple(out_names),
            lowering_input_output_aliases=(),
            sim_require_finite=True,
            sim_require_nnan=True,
            nc=nc,
        )
        return tuple(outs)

    devices = jax.devices()[:N_CORES]
    mesh = Mesh(np.asarray(devices), ("core",))
    n_outs = len(out_names)
    jitted = jax.jit(
        shard_map(
            _body,
            mesh=mesh,
            in_specs=(PartitionSpec("core"),) * (n_params + n_outs),
            out_specs=(PartitionSpec("core"),) * n_outs,
            check_rep=False,
        ),
        keep_unused=True,
    )

    def run(in_maps):
        concat_in = [
            np.concatenate(
                [np.asarray(in_maps[c][nm]) for c in range(N_CORES)], axis=0
            )
            for nm in in_names
        ]
        concat_zeros = [
            np.zeros((N_CORES * a.shape[0], *a.shape[1:]), a.dtype)
            for a in out_avals
        ]
        outs = jitted(*concat_in, *concat_zeros)
        return [
            {
                nm: np.asarray(outs[i]).reshape(N_CORES, *out_avals[i].shape)[c]
                for i, nm in enumerate(out_names)
            }
            for c in range(N_CORES)
        ]

    return run


def kernel(x, W1, b1, W2, b2, phases):
    """Full-input entry point. `phases` is unused: the reference's phase
    assignment is the static contiguous partition of the sequence."""
    global _RUNNER
    x = np.asarray(x, dtype=np.float32)
    W1 = np.asarray(W1, dtype=np.float32)
    b1 = np.asarray(b1, dtype=np.float32)
    W2 = np.asarray(W2, dtype=np.float32)
    b2 = np.asarray(b2, dtype=np.float32)

    if _RUNNER is None:
        _RUNNER = _make_runner()
    in_maps = _shard_inputs(x, W1, b1, W2)
    try:
        results = _RUNNER(in_maps)
    except Exception:
        # transient NRT device errors have been observed; retry once
        results = _RUNNER(in_maps)
    return _unshard_outputs(results, b2)


if __name__ == "__main__":
    rng = np.random.default_rng(0)
    x = rng.standard_normal((B, S, D), dtype=np.float32)
    W1 = (rng.random((P, D, F), dtype=np.float32) - 0.5) / np.sqrt(D)
    b1 = (rng.random((P, F), dtype=np.float32) - 0.5) / np.sqrt(D)
    W2 = (rng.random((P, F, D), dtype=np.float32) - 0.5) / np.sqrt(F)
    b2 = (rng.random((P, D), dtype=np.float32) - 0.5) / np.sqrt(F)
    phases = rng.integers(0, P, size=(B, S)).astype(np.int32)

    y = kernel(x, W1, b1, W2, b2, phases)

    xs = x.reshape(B, P, S // P, D)
    h = np.maximum(np.einsum("bpsd,pdf->bpsf", xs, W1) + b1[None, :, None, :], 0.0)
    yref = (np.einsum("bpsf,pfd->bpsd", h, W2) + b2[None, :, None, :]).reshape(B, S, D)
    err = np.linalg.norm(y - yref) / np.linalg.norm(yref)
    print("rel err:", err)


# revision 19
# speedup vs baseline: 1.0163x; 1.0163x over previous
"""Branched feed-forward (4-phase MoE-style FF) on 8 Trainium2 NeuronCores.

Reference computation (B=32, S=1024, D=1024, P=4, F=4096):
    xs = x.reshape(B, P, S//P, D)              # static contiguous phase split
    h  = relu(xs @ W1[p] + b1[p])              # per-phase FF, D -> F
    y  = h @ W2[p] + b2[p]                     # F -> D
    out = y.reshape(B, S, D)

Sharding: 8 cores = 4 phases x 2 F-halves (expert parallel + FF-width
parallel).  Core c handles phase p = c//2, F-half fh = c%2: it computes a
partial y (contraction over its half of F) for ALL 8192 tokens of its
phase.  Host sums the two partials per phase and adds b2 (cheap numpy).

Per-core kernel (all weights SBUF-resident, bf16 matmuls / fp32 PSUM):
    FF1(tb): h[ft, :] = relu( sum_dc W1c[ft][dc].T @ x[dc, :] + b1[ft] )
    FF2(tb): y[dt, :] = sum_fc W2c[fc][:, dt].T @ h[fc, :]

Weights are DMA'd in 2KB-per-partition chunks (one per 128-wide output
tile) so the first matmuls start as soon as chunk 0 + x-block 0 land,
hiding the ~8MB weight load behind compute instead of serializing it at
NEFF start.  The x block DMA is split across DC chunks to spread it over
multiple DMA queues (a single queue only gets ~1/11 of HBM bandwidth
while the weight chunks stream in).

Measured on this hardware: with >=4 of the 8 cores under sustained PE
load the chip power-throttles the PE to ~2.0GHz (core-count scaling
measured 907/894/1098/1097us per-core at 1/2/4/8 active cores — full
2.4GHz at <=2 cores), so the 8-core per-core floor is 4096 MMs x
512cyc/2.0GHz + ~5ns/instruction x 8192 PE instructions ~ 1.09ms, and
this kernel measures exactly that.  bf16 and fp32r stream at the same
1 cycle/row (fp32r is single-pass at N>=256), so the win over the
fp32r baseline comes from halved LDWEIGHTS / DMA bytes, tt=512 (half
the dispatch overhead), and the overlapped weight load.  All DMAs
share one HW-DGE FIFO ring (qSPDynamicHW), so startup DMA issue order
is strict criticality order.  fp8 DoubleRow (0.5 cyc/row) fails
accuracy: measured 5.0e-2 final rel err vs the 2e-2 gate; the
3-matmul error-split costs 1.7x bf16 cycles.  An FF1(tb+1)/FF2(tb)
skewed pipeline measured 11us WORSE (strict-FIFO ACT queue
head-blocks on the skewed WAR), so the layers run in natural order.

Matmul dtype MM_DT: "bfloat16" (~3.2e-3 rel err on the graded inputs)
vs gate 2e-2.
"""

import numpy as np

import concourse.bacc as bacc
import concourse.mybir as mybir
import concourse.tile as tile
from concourse.bass import ts

# Problem dims (hardcoded per contest contract)
B, S, D = 32, 1024, 1024
P, F = 4, 4096
N_CORES = 8

# Per-core dims
FH = F // 2          # F half per core = 2048
T = B * (S // P)     # tokens per phase = 8192
DC = D // 128        # 8 contraction chunks for FF1 / out tiles for FF2
FT = FH // 128       # 16 out tiles for FF1 / contraction chunks for FF2

# Tunables (defaults = the graded configuration)
MM_DT = "bfloat16"   # matmul dtype: "bfloat16" | "float32r"
TT = 512             # token block (matmul moving free dim)

F32 = mybir.dt.float32


def build_bass(
    reps=1, loop_reps=1, mm_dt=None, tt=None, skew=False, mono=False, wil=False
):
    """Build the per-core Bass program.

    `reps` repeats the compute sweep by instruction duplication; `loop_reps`
    repeats it via a hardware For_i loop (no code growth).  Both are timing
    aids for test.py (slope between rep counts isolates on-device time);
    the graded kernel uses reps=1, loop_reps=1."""
    mm_dt = MM_DT if mm_dt is None else mm_dt
    tt = TT if tt is None else tt
    DT = getattr(mybir.dt, mm_dt)
    tb_n = T // tt

    nc = bacc.Bacc(None, target_bir_lowering=False)

    # Host pre-permutes everything so every DMA line is one contiguous
    # per-partition chunk (x: tt*esz per dc chunk, w1/w2: 2KB, y: tt*4B).
    x_d = nc.dram_tensor("x", [tb_n, 128, DC, tt], DT, kind="ExternalInput")
    w1_d = nc.dram_tensor("w1", [FT, 128, DC, 128], DT, kind="ExternalInput")
    w2_d = nc.dram_tensor("w2", [FT, 128, D], DT, kind="ExternalInput")
    b1_d = nc.dram_tensor("b1", [128, FT], F32, kind="ExternalInput")
    y_d = nc.dram_tensor("y", [tb_n, DC, 128, tt], F32, kind="ExternalOutput")

    with tile.TileContext(nc) as tc:
        with (
            tc.tile_pool(name="weights", bufs=1) as wpool,
            tc.tile_pool(name="xin", bufs=3) as xpool,
            tc.tile_pool(name="hbuf", bufs=2) as hpool,
            tc.tile_pool(name="yout", bufs=4) as ypool,
            tc.tile_pool(name="psum", bufs=8, space="PSUM") as psum,
        ):
            x_first = {}
            if mono:
                # Baseline-like dependency structure: one tile per weight
                # matrix, one DMA per x block (strided gather per partition).
                w1all = wpool.tile([128, FT, DC, 128], DT, tag="w1")
                nc.sync.dma_start(w1all[:], w1_d.transpose([1, 0, 2, 3]))
                w2all = wpool.tile([128, FT, D], DT, tag="w2")
                nc.sync.dma_start(w2all[:], w2_d.transpose([1, 0, 2]))
                b1_s = wpool.tile([128, FT], F32, tag="b1")
                nc.sync.dma_start(b1_s[:], b1_d[:])
                w1c = [w1all[:, ft] for ft in range(FT)]
                w2c = [w2all[:, fc] for fc in range(FT)]
            else:
                xf0 = wpool.tile([128, DC, tt], DT, tag="x0", name="xf0")
                x_first[0] = xf0
                w1c = [
                    wpool.tile([128, DC, 128], DT, tag=f"w1_{ft}", name=f"w1c{ft}")
                    for ft in range(FT)
                ]
                if tb_n > 1:
                    xf1 = wpool.tile([128, DC, tt], DT, tag="x1", name="xf1")
                    x_first[1] = xf1
                b1_s = wpool.tile([128, FT], F32, tag="b1")
                w2c = [
                    wpool.tile([128, D], DT, tag=f"w2_{fc}", name=f"w2c{fc}")
                    for fc in range(FT)
                ]

            def load_weights():
                # All DMAs share one HW-DGE FIFO ring (qSPDynamicHW), so
                # issue order IS arrival order at ~358GB/s: strict
                # criticality. x(0) + w1 chunk 0 feed the first matmul group
                # (~4us in); w1 chunks 1-3 beat FF1(0)'s consumption rate;
                # x(1)/b1/the rest arrive well before they're needed.
                for dc in range(DC):
                    nc.sync.dma_start(x_first[0][:, dc, :], x_d[0, :, dc, :])
                for dc in range(DC):
                    nc.sync.dma_start(w1c[0][:, dc, :], w1_d[0, :, dc, :])
                for ft in range(1, 4):
                    nc.sync.dma_start(w1c[ft][:], w1_d[ft])
                if 1 in x_first:
                    for dc in range(DC):
                        nc.sync.dma_start(x_first[1][:, dc, :], x_d[1, :, dc, :])
                nc.sync.dma_start(b1_s[:], b1_d[:])
                for ft in range(4, FT):
                    nc.sync.dma_start(w1c[ft][:], w1_d[ft])
                for fc in range(FT):
                    nc.sync.dma_start(w2c[fc][:], w2_d[fc])

            def ff2(h_t, tb):
                for dt_ in range(DC):
                    ps = psum.tile([128, tt], F32, tag="ps")
                    for fc in range(FT):
                        nc.tensor.matmul(
                            ps[:],
                            w2c[fc][:, ts(dt_, 128)],
                            h_t[:, fc, :],
                            start=(fc == 0),
                            stop=(fc == FT - 1),
                        )
                    y_t = ypool.tile([128, tt], F32, tag="y")
                    nc.vector.tensor_copy(y_t[:], ps[:])
                    nc.sync.dma_start(y_d[tb, dt_], y_t[:])

            def sweep():
                prev = None
                for tb in [t for _ in range(reps) for t in range(tb_n)]:
                    if tb in x_first:
                        x_t = x_first[tb]
                    elif mono:
                        x_t = xpool.tile([128, DC, tt], DT, tag="x")
                        nc.sync.dma_start(x_t[:], x_d[tb])
                    else:
                        x_t = xpool.tile([128, DC, tt], DT, tag="x")
                        for dc in range(DC):
                            nc.sync.dma_start(x_t[:, dc, :], x_d[tb, :, dc, :])

                    h_t = hpool.tile([128, FT, tt], DT, tag="h")
                    for ft in range(FT):
                        ps = psum.tile([128, tt], F32, tag="ps")
                        for dc in range(DC):
                            nc.tensor.matmul(
                                ps[:],
                                w1c[ft][:, dc, :],
                                x_t[:, dc, :],
                                start=(dc == 0),
                                stop=(dc == DC - 1),
                            )
                        nc.scalar.activation(
                            h_t[:, ft, :],
                            ps[:],
                            mybir.ActivationFunctionType.Relu,
                            bias=b1_s[:, ft : ft + 1],
                        )
                    if skew:
                        if prev is not None:
                            ff2(*prev)
                        prev = (h_t, tb)
                    else:
                        ff2(h_t, tb)
                if skew:
                    ff2(*prev)

            if loop_reps > 1 and wil:
                assert not mono
                with tc.For_i(0, loop_reps, 1):
                    load_weights()
                    sweep()
            elif loop_reps > 1:
                if not mono:
                    load_weights()
                with tc.For_i(0, loop_reps, 1):
                    sweep()
            else:
                if not mono:
                    load_weights()
                sweep()

    nc.compile()
    return nc


def _np_dt(mm_dt=None):
    return mybir.dt.np(getattr(mybir.dt, MM_DT if mm_dt is None else mm_dt))


def _shard_inputs(x, W1, b1, W2, mm_dt=None, tt=None):
    """Build the 8 per-core input maps. Core c: phase c//2, F-half c%2."""
    tt = TT if tt is None else tt
    tb_n = T // tt
    np_dt = _np_dt(mm_dt)
    in_maps = []
    xt_by_phase = {}
    for c in range(N_CORES):
        p, fh = divmod(c, 2)
        if p not in xt_by_phase:  # both F-half cores of a phase share x
            xs = np.ascontiguousarray(x.reshape(B, P, S // P, D)[:, p])
            xt_by_phase[p] = np.ascontiguousarray(
                xs.reshape(tb_n, tt, DC, 128).transpose(0, 3, 2, 1)  # [tbn,128,DC,tt]
            ).astype(np_dt)
        xt = xt_by_phase[p]
        # w1: [FT, 128, DC, 128]; w1[ft, part, dc, j] = W1[p][dc*128+part, fh*FH+ft*128+j]
        w1 = (
            W1[p][:, fh * FH : (fh + 1) * FH]
            .reshape(DC, 128, FT, 128)
            .transpose(2, 1, 0, 3)
        )
        # w2: [FT, 128, D]; w2[fc, part, :] = W2[p][fh*FH+fc*128+part, :]
        w2 = W2[p][fh * FH : (fh + 1) * FH, :].reshape(FT, 128, D)
        b1c = b1[p][fh * FH : (fh + 1) * FH].reshape(FT, 128).T
        in_maps.append(
            {
                "x": xt,
                "w1": np.ascontiguousarray(w1).astype(np_dt),
                "w2": np.ascontiguousarray(w2).astype(np_dt),
                "b1": np.ascontiguousarray(b1c).astype(np.float32),
            }
        )
    return in_maps


def _unshard_outputs(results, b2, tt=None):
    """results: list of 8 dicts with 'y' [tb_n,DC,128,tt] partial sums."""
    tt = TT if tt is None else tt
    y = np.empty((B, P, S // P, D), dtype=np.float32)
    for p in range(P):
        ya = results[2 * p]["y"]
        yb = results[2 * p + 1]["y"]
        # [tbn,DC,128,tt] -> [tbn,tt,DC,128] -> [T, D]
        yp = (ya + yb).transpose(0, 3, 1, 2).reshape(T, D) + b2[p][None, :]
        y[:, p] = yp.reshape(B, S // P, D)
    return y.reshape(B, S, D)


# ---------------------------------------------------------------------------
# Compile-once PJRT runner (mirrors concourse.bass2jax.run_bass_via_pjrt but
# caches the sharded executable so repeat kernel() calls skip re-tracing).

_RUNNER = None


def _make_runner():
    import jax
    from jax.sharding import Mesh, PartitionSpec
    from jax.experimental.shard_map import shard_map
    from concourse.bass2jax import (
        _bass_exec_p,
        install_neuronx_cc_hook,
        partition_id_tensor,
    )

    nc = build_bass()
    install_neuronx_cc_hook()

    partition_name = nc.partition_id_tensor.name if nc.partition_id_tensor else None

    in_names, out_names, out_avals = [], [], []
    for alloc in nc.m.functions[0].allocations:
        if not isinstance(alloc, mybir.MemoryLocationSet):
            continue
        name = alloc.memorylocations[0].name
        if alloc.kind == "ExternalInput":
            if name != partition_name:
                in_names.append(name)
        elif alloc.kind == "ExternalOutput":
            out_names.append(name)
            out_avals.append(
                jax.core.ShapedArray(
                    tuple(alloc.tensor_shape), mybir.dt.np(alloc.dtype)
                )
            )
    n_params = len(in_names)
    all_in_names = list(in_names) + list(out_names)
    if partition_name is not None:
        all_in_names.append(partition_name)

    def _body(*args):
        operands = list(args)
        if partition_name is not None:
            operands.append(partition_id_tensor())
        outs = _bass_exec_p.bind(
            *operands,
            out_avals=tuple(out_avals),
            in_names=tuple(all_in_names),
            out_names=tuple(out_names),
            lowering_input_output_aliases=(),
            sim_require_finite=True,
            sim_require_nnan=True,
            nc=nc,
        )
        return tuple(outs)

    devices = jax.devices()[:N_CORES]
    mesh = Mesh(np.asarray(devices), ("core",))
    n_outs = len(out_names)
    jitted = jax.jit(
        shard_map(
            _body,
            mesh=mesh,
            in_specs=(PartitionSpec("core"),) * (n_params + n_outs),
            out_specs=(PartitionSpec("core"),) * n_outs,
            check_rep=False,
        ),
        keep_unused=True,
    )

    def run(in_maps):
        concat_in = [
            np.concatenate(
                [np.asarray(in_maps[c][nm]) for c in range(N_CORES)], axis=0
            )
            for nm in in_names
        ]
        concat_zeros = [
            np.zeros((N_CORES * a.shape[0], *a.shape[1:]), a.dtype)
            for a in out_avals
        ]
        outs = jitted(*concat_in, *concat_zeros)
        return [
            {
                nm: np.asarray(outs[i]).reshape(N_CORES, *out_avals[i].shape)[c]
                for i, nm in enumerate(out_names)
            }
            for c in range(N_CORES)
        ]

    return run


def kernel(x, W1, b1, W2, b2, phases):
    """Full-input entry point. `phases` is unused: the reference's phase
    assignment is the static contiguous partition of the sequence."""
    global _RUNNER
    x = np.asarray(x, dtype=np.float32)
    W1 = np.asarray(W1, dtype=np.float32)
    b1 = np.asarray(b1, dtype=np.float32)
    W2 = np.asarray(W2, dtype=np.float32)
    b2 = np.asarray(b2, dtype=np.float32)

    if _RUNNER is None:
        _RUNNER = _make_runner()
    in_maps = _shard_inputs(x, W1, b1, W2)
    try:
        results = _RUNNER(in_maps)
    except Exception:
        # transient NRT device errors have been observed; retry once
        results = _RUNNER(in_maps)
    return _unshard_outputs(results, b2)


if __name__ == "__main__":
    rng = np.random.default_rng(0)
    x = rng.standard_normal((B, S, D), dtype=np.float32)
    W1 = (rng.random((P, D, F), dtype=np.float32) - 0.5) / np.sqrt(D)
    b1 = (rng.random((P, F), dtype=np.float32) - 0.5) / np.sqrt(D)
    W2 = (rng.random((P, F, D), dtype=np.float32) - 0.5) / np.sqrt(F)
    b2 = (rng.random((P, D), dtype=np.float32) - 0.5) / np.sqrt(F)
    phases = rng.integers(0, P, size=(B, S)).astype(np.int32)

    y = kernel(x, W1, b1, W2, b2, phases)

    xs = x.reshape(B, P, S // P, D)
    h = np.maximum(np.einsum("bpsd,pdf->bpsf", xs, W1) + b1[None, :, None, :], 0.0)
    yref = (np.einsum("bpsf,pfd->bpsd", h, W2) + b2[None, :, None, :]).reshape(B, S, D)
    err = np.linalg.norm(y - yref) / np.linalg.norm(yref)
    print("rel err:", err)


# revision 24
# speedup vs baseline: 1.1601x; 1.1415x over previous
"""Branched feed-forward (4-phase MoE-style FF) on 8 Trainium2 NeuronCores.

Reference computation (B=32, S=1024, D=1024, P=4, F=4096):
    xs = x.reshape(B, P, S//P, D)              # static contiguous phase split
    h  = relu(xs @ W1[p] + b1[p])              # per-phase FF, D -> F
    y  = h @ W2[p] + b2[p]                     # F -> D
    out = y.reshape(B, S, D)

Sharding: 8 cores = 4 phases x 2 F-halves (expert parallel + FF-width
parallel).  Core c handles phase p = c//2, F-half fh = c%2: it computes a
partial y (contraction over its half of F) for ALL 8192 tokens of its
phase.  Host sums the two partials per phase and adds b2 (cheap numpy).

Per-core kernel (all weights SBUF-resident, bf16 matmuls / fp32 PSUM):
    FF1(tb): h[ft, :] = relu( sum_dc W1c[ft][dc].T @ x[dc, :] + b1[ft] )
    FF2(tb): y[dt, :] = sum_fc W2c[fc][:, dt].T @ h[fc, :]

Weights are DMA'd in 2KB-per-partition chunks (one per 128-wide output
tile) so the first matmuls start as soon as chunk 0 + x-block 0 land,
hiding the ~8MB weight load behind compute instead of serializing it at
NEFF start.  The x block DMA is split across DC chunks to spread it over
multiple DMA queues (a single queue only gets ~1/11 of HBM bandwidth
while the weight chunks stream in).

Measured on this hardware: with >=4 of the 8 cores under sustained PE
load the chip power-throttles the PE to ~2.0GHz (core-count scaling
measured 907/894/1098/1097us per-core at 1/2/4/8 active cores — full
2.4GHz at <=2 cores), so the 8-core per-core floor is 4096 MMs x
512cyc/2.0GHz + ~5ns/instruction x 8192 PE instructions ~ 1.09ms, and
this kernel measures exactly that.  bf16 and fp32r stream at the same
1 cycle/row (fp32r is single-pass at N>=256), so the win over the
fp32r baseline comes from halved LDWEIGHTS / DMA bytes, tt=512 (half
the dispatch overhead), and the overlapped weight load.  All DMAs
share one HW-DGE FIFO ring (qSPDynamicHW), so startup DMA issue order
is strict criticality order.  fp8 DoubleRow (0.5 cyc/row) fails
accuracy: measured 5.0e-2 final rel err vs the 2e-2 gate; the
3-matmul error-split costs 1.7x bf16 cycles.  An FF1(tb+1)/FF2(tb)
skewed pipeline measured 11us WORSE (strict-FIFO ACT queue
head-blocks on the skewed WAR), so the layers run in natural order.

Matmul dtype MM_DT: "bfloat16" (~3.2e-3 rel err on the graded inputs)
vs gate 2e-2.
"""

import numpy as np

import concourse.bacc as bacc
import concourse.mybir as mybir
import concourse.tile as tile
from concourse.bass import ts

# Problem dims (hardcoded per contest contract)
B, S, D = 32, 1024, 1024
P, F = 4, 4096
N_CORES = 8

# Per-core dims
FH = F // 2          # F half per core = 2048
T = B * (S // P)     # tokens per phase = 8192
DC = D // 128        # 8 contraction chunks for FF1 / out tiles for FF2
FT = FH // 128       # 16 out tiles for FF1 / contraction chunks for FF2

# Tunables (defaults = the graded configuration)
MM_DT = "bfloat16"   # matmul dtype: "bfloat16" | "float32r"
TT = 512             # token block (matmul moving free dim)

F32 = mybir.dt.float32


def build_bass(
    reps=1,
    loop_reps=1,
    mm_dt=None,
    tt=None,
    skew=False,
    mono=False,
    wil=False,
    bare=False,
):
    """Build the per-core Bass program.

    `reps` repeats the compute sweep by instruction duplication; `loop_reps`
    repeats it via a hardware For_i loop (no code growth).  Both are timing
    aids for test.py (slope between rep counts isolates on-device time);
    the graded kernel uses reps=1, loop_reps=1."""
    mm_dt = MM_DT if mm_dt is None else mm_dt
    tt = TT if tt is None else tt
    DT = getattr(mybir.dt, mm_dt)
    tb_n = T // tt

    nc = bacc.Bacc(None, target_bir_lowering=False)

    # Host pre-permutes everything so every DMA line is one contiguous
    # per-partition chunk (x: tt*esz per dc chunk, w1/w2: 2KB, y: tt*4B).
    x_d = nc.dram_tensor("x", [tb_n, 128, DC, tt], DT, kind="ExternalInput")
    w1_d = nc.dram_tensor("w1", [FT, 128, DC, 128], DT, kind="ExternalInput")
    w2_d = nc.dram_tensor("w2", [FT, 128, D], DT, kind="ExternalInput")
    b1_d = nc.dram_tensor("b1", [128, FT], F32, kind="ExternalInput")
    y_d = nc.dram_tensor("y", [tb_n, DC, 128, tt], F32, kind="ExternalOutput")

    with tile.TileContext(nc) as tc:
        with (
            tc.tile_pool(name="weights", bufs=1) as wpool,
            tc.tile_pool(name="xin", bufs=3) as xpool,
            tc.tile_pool(name="hbuf", bufs=2) as hpool,
            tc.tile_pool(name="yout", bufs=4) as ypool,
            tc.tile_pool(name="psum", bufs=8, space="PSUM") as psum,
        ):
            x_first = {}
            if mono:
                # Baseline-like dependency structure: one tile per weight
                # matrix, one DMA per x block (strided gather per partition).
                w1all = wpool.tile([128, FT, DC, 128], DT, tag="w1")
                nc.sync.dma_start(w1all[:], w1_d.transpose([1, 0, 2, 3]))
                w2all = wpool.tile([128, FT, D], DT, tag="w2")
                nc.sync.dma_start(w2all[:], w2_d.transpose([1, 0, 2]))
                b1_s = wpool.tile([128, FT], F32, tag="b1")
                nc.sync.dma_start(b1_s[:], b1_d[:])
                w1c = [w1all[:, ft] for ft in range(FT)]
                w2c = [w2all[:, fc] for fc in range(FT)]
            else:
                xf0 = wpool.tile([128, DC, tt], DT, tag="x0", name="xf0")
                x_first[0] = xf0
                w1c = [
                    wpool.tile([128, DC, 128], DT, tag=f"w1_{ft}", name=f"w1c{ft}")
                    for ft in range(FT)
                ]
                if tb_n > 1:
                    xf1 = wpool.tile([128, DC, tt], DT, tag="x1", name="xf1")
                    x_first[1] = xf1
                b1_s = wpool.tile([128, FT], F32, tag="b1")
                w2c = [
                    wpool.tile([128, D], DT, tag=f"w2_{fc}", name=f"w2c{fc}")
                    for fc in range(FT)
                ]

            def load_weights():
                # All DMAs share one HW-DGE FIFO ring (qSPDynamicHW), so
                # issue order IS arrival order at ~358GB/s: strict
                # criticality. x(0) + w1 chunk 0 feed the first matmul group
                # (~4us in); w1 chunks 1-3 beat FF1(0)'s consumption rate;
                # x(1)/b1/the rest arrive well before they're needed.
                for dc in range(DC):
                    nc.sync.dma_start(x_first[0][:, dc, :], x_d[0, :, dc, :])
                for dc in range(DC):
                    nc.sync.dma_start(w1c[0][:, dc, :], w1_d[0, :, dc, :])
                for ft in range(1, 4):
                    nc.sync.dma_start(w1c[ft][:], w1_d[ft])
                if 1 in x_first:
                    for dc in range(DC):
                        nc.sync.dma_start(x_first[1][:, dc, :], x_d[1, :, dc, :])
                nc.sync.dma_start(b1_s[:], b1_d[:])
                for ft in range(4, FT):
                    nc.sync.dma_start(w1c[ft][:], w1_d[ft])
                for fc in range(FT):
                    nc.sync.dma_start(w2c[fc][:], w2_d[fc])

            h_bare = None
            if bare:
                h_bare = wpool.tile([128, FT, tt], DT, tag="hbare")
                nc.vector.memset(h_bare[:], 0.25)

            def ff2(h_t, tb):
                for dt_ in range(DC):
                    ps = psum.tile([128, tt], F32, tag="ps")
                    for fc in range(FT):
                        nc.tensor.matmul(
                            ps[:],
                            w2c[fc][:, ts(dt_, 128)],
                            h_t[:, fc, :],
                            start=(fc == 0),
                            stop=(fc == FT - 1),
                        )
                    if bare:
                        continue
                    y_t = ypool.tile([128, tt], F32, tag="y")
                    nc.vector.tensor_copy(y_t[:], ps[:])
                    nc.sync.dma_start(y_d[tb, dt_], y_t[:])

            def sweep():
                prev = None
                for tb in [t for _ in range(reps) for t in range(tb_n)]:
                    if tb in x_first:
                        x_t = x_first[tb]
                    elif mono:
                        x_t = xpool.tile([128, DC, tt], DT, tag="x")
                        nc.sync.dma_start(x_t[:], x_d[tb])
                    else:
                        x_t = xpool.tile([128, DC, tt], DT, tag="x")
                        for dc in range(DC):
                            nc.sync.dma_start(x_t[:, dc, :], x_d[tb, :, dc, :])

                    h_t = h_bare if bare else hpool.tile([128, FT, tt], DT, tag="h")
                    for ft in range(FT):
                        ps = psum.tile([128, tt], F32, tag="ps")
                        for dc in range(DC):
                            nc.tensor.matmul(
                                ps[:],
                                w1c[ft][:, dc, :],
                                x_t[:, dc, :],
                                start=(dc == 0),
                                stop=(dc == DC - 1),
                            )
                        if not bare:
                            nc.scalar.activation(
                                h_t[:, ft, :],
                                ps[:],
                                mybir.ActivationFunctionType.Relu,
                                bias=b1_s[:, ft : ft + 1],
                            )
                    if skew:
                        if prev is not None:
                            ff2(*prev)
                        prev = (h_t, tb)
                    else:
                        ff2(h_t, tb)
                if skew:
                    ff2(*prev)

            if loop_reps > 1 and wil:
                assert not mono
                with tc.For_i(0, loop_reps, 1):
                    load_weights()
                    sweep()
            elif loop_reps > 1:
                if not mono:
                    load_weights()
                with tc.For_i(0, loop_reps, 1):
                    sweep()
            else:
                if not mono:
                    load_weights()
                sweep()

    nc.compile()
    return nc


def _np_dt(mm_dt=None):
    return mybir.dt.np(getattr(mybir.dt, MM_DT if mm_dt is None else mm_dt))


def _shard_inputs(x, W1, b1, W2, mm_dt=None, tt=None):
    """Build the 8 per-core input maps. Core c: phase c//2, F-half c%2."""
    tt = TT if tt is None else tt
    tb_n = T // tt
    np_dt = _np_dt(mm_dt)
    in_maps = []
    xt_by_phase = {}
    for c in range(N_CORES):
        p, fh = divmod(c, 2)
        if p not in xt_by_phase:  # both F-half cores of a phase share x
            xs = np.ascontiguousarray(x.reshape(B, P, S // P, D)[:, p])
            xt_by_phase[p] = np.ascontiguousarray(
                xs.reshape(tb_n, tt, DC, 128).transpose(0, 3, 2, 1)  # [tbn,128,DC,tt]
            ).astype(np_dt)
        xt = xt_by_phase[p]
        # w1: [FT, 128, DC, 128]; w1[ft, part, dc, j] = W1[p][dc*128+part, fh*FH+ft*128+j]
        w1 = (
            W1[p][:, fh * FH : (fh + 1) * FH]
            .reshape(DC, 128, FT, 128)
            .transpose(2, 1, 0, 3)
        )
        # w2: [FT, 128, D]; w2[fc, part, :] = W2[p][fh*FH+fc*128+part, :]
        w2 = W2[p][fh * FH : (fh + 1) * FH, :].reshape(FT, 128, D)
        b1c = b1[p][fh * FH : (fh + 1) * FH].reshape(FT, 128).T
        in_maps.append(
            {
                "x": xt,
                "w1": np.ascontiguousarray(w1).astype(np_dt),
                "w2": np.ascontiguousarray(w2).astype(np_dt),
                "b1": np.ascontiguousarray(b1c).astype(np.float32),
            }
        )
    return in_maps


def _unshard_outputs(results, b2, tt=None):
    """results: list of 8 dicts with 'y' [tb_n,DC,128,tt] partial sums."""
    tt = TT if tt is None else tt
    y = np.empty((B, P, S // P, D), dtype=np.float32)
    for p in range(P):
        ya = results[2 * p]["y"]
        yb = results[2 * p + 1]["y"]
        # [tbn,DC,128,tt] -> [tbn,tt,DC,128] -> [T, D]
        yp = (ya + yb).transpose(0, 3, 1, 2).reshape(T, D) + b2[p][None, :]
        y[:, p] = yp.reshape(B, S // P, D)
    return y.reshape(B, S, D)


# ---------------------------------------------------------------------------
# Compile-once PJRT runner (mirrors concourse.bass2jax.run_bass_via_pjrt but
# caches the sharded executable so repeat kernel() calls skip re-tracing).

_RUNNER = None


def _make_runner():
    import jax
    from jax.sharding import Mesh, PartitionSpec
    from jax.experimental.shard_map import shard_map
    from concourse.bass2jax import (
        _bass_exec_p,
        install_neuronx_cc_hook,
        partition_id_tensor,
    )

    nc = build_bass()
    install_neuronx_cc_hook()

    partition_name = nc.partition_id_tensor.name if nc.partition_id_tensor else None

    in_names, out_names, out_avals = [], [], []
    for alloc in nc.m.functions[0].allocations:
        if not isinstance(alloc, mybir.MemoryLocationSet):
            continue
        name = alloc.memorylocations[0].name
        if alloc.kind == "ExternalInput":
            if name != partition_name:
                in_names.append(name)
        elif alloc.kind == "ExternalOutput":
            out_names.append(name)
            out_avals.append(
                jax.core.ShapedArray(
                    tuple(alloc.tensor_shape), mybir.dt.np(alloc.dtype)
                )
            )
    n_params = len(in_names)
    all_in_names = list(in_names) + list(out_names)
    if partition_name is not None:
        all_in_names.append(partition_name)

    def _body(*args):
        operands = list(args)
        if partition_name is not None:
            operands.append(partition_id_tensor())
        outs = _bass_exec_p.bind(
            *operands,
            out_avals=tuple(out_avals),
            in_names=tuple(all_in_names),
            out_names=tuple(out_names),
            lowering_input_output_aliases=(),
            sim_require_finite=True,
            sim_require_nnan=True,
            nc=nc,
        )
        return tuple(outs)

    devices = jax.devices()[:N_CORES]
    mesh = Mesh(np.asarray(devices), ("core",))
    n_outs = len(out_names)
    jitted = jax.jit(
        shard_map(
            _body,
            mesh=mesh,
            in_specs=(PartitionSpec("core"),) * (n_params + n_outs),
            out_specs=(PartitionSpec("core"),) * n_outs,
            check_rep=False,
        ),
        keep_unused=True,
    )

    def run(in_maps):
        concat_in = [
            np.concatenate(
                [np.asarray(in_maps[c][nm]) for c in range(N_CORES)], axis=0
            )
            for nm in in_names
        ]
        concat_zeros = [
            np.zeros((N_CORES * a.shape[0], *a.shape[1:]), a.dtype)
            for a in out_avals
        ]
        outs = jitted(*concat_in, *concat_zeros)
        return [
            {
                nm: np.asarray(outs[i]).reshape(N_CORES, *out_avals[i].shape)[c]
                for i, nm in enumerate(out_names)
            }
            for c in range(N_CORES)
        ]

    return run


def kernel(x, W1, b1, W2, b2, phases):
    """Full-input entry point. `phases` is unused: the reference's phase
    assignment is the static contiguous partition of the sequence."""
    global _RUNNER
    x = np.asarray(x, dtype=np.float32)
    W1 = np.asarray(W1, dtype=np.float32)
    b1 = np.asarray(b1, dtype=np.float32)
    W2 = np.asarray(W2, dtype=np.float32)
    b2 = np.asarray(b2, dtype=np.float32)

    if _RUNNER is None:
        _RUNNER = _make_runner()
    in_maps = _shard_inputs(x, W1, b1, W2)
    try:
        results = _RUNNER(in_maps)
    except Exception:
        # transient NRT device errors have been observed; retry once
        results = _RUNNER(in_maps)
    return _unshard_outputs(results, b2)


if __name__ == "__main__":
    rng = np.random.default_rng(0)
    x = rng.standard_normal((B, S, D), dtype=np.float32)
    W1 = (rng.random((P, D, F), dtype=np.float32) - 0.5) / np.sqrt(D)
    b1 = (rng.random((P, F), dtype=np.float32) - 0.5) / np.sqrt(D)
    W2 = (rng.random((P, F, D), dtype=np.float32) - 0.5) / np.sqrt(F)
    b2 = (rng.random((P, D), dtype=np.float32) - 0.5) / np.sqrt(F)
    phases = rng.integers(0, P, size=(B, S)).astype(np.int32)

    y = kernel(x, W1, b1, W2, b2, phases)

    xs = x.reshape(B, P, S // P, D)
    h = np.maximum(np.einsum("bpsd,pdf->bpsf", xs, W1) + b1[None, :, None, :], 0.0)
    yref = (np.einsum("bpsf,pfd->bpsd", h, W2) + b2[None, :, None, :]).reshape(B, S, D)
    err = np.linalg.norm(y - yref) / np.linalg.norm(yref)
    print("rel err:", err)
